# revision 24
# baseline (speedup 1.0000x reference)
"""Dense MLP forward (y = quantize(relu(x @ w + b))) on 8 TRN2 NeuronCores.

Strategy: pure data-parallel over the batch dim (1024 rows per core), w/b
replicated, no collectives. Host-side each core receives its x shard
transposed AND pre-blocked into contiguous [128, 512] DMA tiles, in fp16
(x and w are fixed-point values; fp16 matmul keeps rel-err ~3e-4, far under
the 2e-2 gate, and halves input HBM traffic vs fp32). Each core computes yT:

  - 128 matmuls of [128k,128n] stationary x [128k,512m] moving in fp16
    (1 cycle/row at 2.4GHz warm => ~216ns each), accumulating over the 8
    k-chunks into all 8 PSUM banks.
  - chunk 0 (the tiny [128,128] w slice the first LDWEIGHTS needs, the first
    x piece, and the rest of w chunk 0) ships via SWDGE (gpsimd) whose queue
    starts independently of the two HWDGE rings; remaining chunk pieces
    alternate between the SP and ACT HWDGE rings so each chunk's w+x pair
    lands in ~1.1us. The tiny first transfers double as SDMA wake-ups
    (engine 15 otherwise wakes ~1.7us late and every sem_increment=16 wait
    eats that latency).
  - band 0 (m=0:512): k-major waves -- 8 MMs per arriving k-chunk, so the
    PE starts as soon as chunk 0 lands and never outruns DMA. Groups run in
    order (7,0,1,...,6) within each wave, so group 7's accumulation stops
    first and its PSUM bank is free well before band 1 reuses it.
  - band 1 (m=512:1024): per-tile k-bursts in the same rotated order; group
    stops stagger 1.7us apart so DVE evictions + output DMAs overlap the
    MMs, every burst's bank is long-evicted when it starts (no
    write-after-read stalls), and the last burst's eviction is split into
    two 256-wide pieces on alternating rings to shorten the final
    epilogue + store chain.
  - epilogue per tile: relu(psum + b) in one DVE tensor_scalar op (bias is
    per-partition in the transposed layout), output in bf16 (rel err ~2e-3,
    still 10x under the gate) halving output traffic and the store tail.
    No ACT activations are used so the ~1.3us ACT table load never gets
    emitted.
  - junk fp16 matmuls on an uninitialized raw SBUF tensor (Tile-untracked,
    so the PE's first LDWEIGHTS has zero dependencies) release the PE HAM
    clock throttle (1.2 -> 2.4 GHz) while the first input DMAs stream in.

The reference's final 2^-16 snap is omitted: bf16 output rounding (~2e-3
rel) dwarfs the quantization grid (~8e-6 rel). Host reassembles the output
blocks and concatenates across cores.
"""

import numpy as np

import concourse.bacc as bacc
import concourse.tile as tile
from concourse import mybir
from concourse.bass_utils import run_bass_kernel_spmd

P = 128
B, D_IN, D_OUT = 8192, 1024, 1024
N_CORES = 8
M = B // N_CORES          # batch rows per core
KC = D_IN // P            # 8 k-chunks
NT = D_OUT // P           # 8 n-groups (PSUM partition tiles)
MB = 512                  # matmul moving free dim / PSUM bank width (fp32)
NUM_MB = M // MB          # 2 m-bands per core

N_WARMUP_MM = 13          # PE HAM warm-up matmuls on junk data
WARM_N = 256              # free dim of warm-up matmuls

# group processing order: 7 first so its bank frees earliest, 6 last
GORDER = [7, 0, 1, 2, 3, 4, 5, 6]

F32 = mybir.dt.float32
F16 = mybir.dt.float16
BF16 = mybir.dt.bfloat16

_CACHE = {}


def build_bass():
    nc = bacc.Bacc("TRN2", target_bir_lowering=False, debug=False)

    # x pre-blocked on host: xb[c, mb] is the contiguous [128, 512] fp16
    # tile for k-chunk c, m-band mb.
    xb_d = nc.dram_tensor("xb", [KC, NUM_MB, P, MB], F16, kind="ExternalInput")
    w_d = nc.dram_tensor("w", [D_IN, D_OUT], F16, kind="ExternalInput")
    # bias pre-arranged on host: b_pc[p, c] = b[c*128 + p]
    b_d = nc.dram_tensor("b", [P, NT], F32, kind="ExternalInput")
    # output blocked the same way: yb[nt, mb] = yT[128nt:128(nt+1), 512mb:...]
    yb_d = nc.dram_tensor("yb", [NT, NUM_MB, P, MB], BF16, kind="ExternalOutput")

    with tile.TileContext(nc) as tc:
        with (
            nc.sbuf_tensor([P, WARM_N], F16) as zt_raw,
            tc.tile_pool(name="const", bufs=1) as cst,
            tc.tile_pool(name="wx", bufs=1) as wx,
            tc.tile_pool(name="outp", bufs=8) as outp,
            tc.tile_pool(name="ps", bufs=1, space="PSUM") as ps,
        ):
            # PE warm-up matmuls on uninitialized junk (values never used;
            # the PSUM bank is overwritten with start=True later).
            zt = zt_raw.ap()
            warm_ps = ps.tile([P, WARM_N], F32, tag="acc7")
            for _ in range(N_WARMUP_MM):
                nc.tensor.matmul(warm_ps, zt[:, :P], zt, start=True, stop=True)

            w_tiles = [wx.tile([P, D_OUT], F16, tag=f"wc{c}", name=f"wc{c}") for c in range(KC)]
            x_tiles = [wx.tile([P, M], F16, tag=f"xc{c}", name=f"xc{c}") for c in range(KC)]
            b_sb = cst.tile([P, NT], F32, tag="bias_raw")

            # ---- input DMA schedule ----
            # The first LDWEIGHTS only needs the tiny w0 g7 slice; it leads
            # the SP ring (and doubles as the SDMA wake-up) while the first
            # x00 half leads the ACT ring. The rest of chunk 0 is split
            # across both rings so the whole chunk lands ~1us sooner.
            g0 = GORDER[0] * P
            nc.sync.dma_start(out=w_tiles[0][:, g0 : g0 + P], in_=w_d.ap()[0:P, g0 : g0 + P])
            nc.scalar.dma_start(out=x_tiles[0][:, :MB], in_=xb_d.ap()[0, 0])
            nc.sync.dma_start(out=w_tiles[0][:, :g0], in_=w_d.ap()[0:P, :g0])
            for c in range(1, KC):
                wr = nc.sync if c % 2 == 1 else nc.scalar
                xr = nc.scalar if c % 2 == 1 else nc.sync
                wr.dma_start(out=w_tiles[c], in_=w_d.ap()[c * P : (c + 1) * P, :])
                xr.dma_start(out=x_tiles[c][:, :MB], in_=xb_d.ap()[c, 0])
                if c == 3:
                    # bias, pre-arranged on host to [p, c] with n = c*128 + p
                    # (per-partition bias in the transposed layout); needed
                    # only by the first eviction.
                    nc.scalar.dma_start(out=b_sb, in_=b_d.ap())
            # band-1 x pieces; land well before band 1 starts.
            for c in range(KC):
                (nc.sync if c % 2 == 0 else nc.scalar).dma_start(
                    out=x_tiles[c][:, MB:], in_=xb_d.ap()[c, 1]
                )

            def emit_mm(acc, mb, nt, c, msl=slice(0, MB), **kw):
                nc.tensor.matmul(
                    acc[:, msl],
                    w_tiles[c][:, nt * P : (nt + 1) * P],
                    x_tiles[c][:, mb * MB : (mb + 1) * MB][:, msl],
                    **kw,
                )

            def evict(acc, mb, nt, ring, msl=slice(0, MB)):
                o = outp.tile([P, MB], BF16, tag="otile")
                # relu(y + b) in one op; bias varies along partitions here.
                nc.vector.tensor_scalar(
                    o[:, msl],
                    acc[:, msl],
                    b_sb[:, nt : nt + 1],
                    0.0,
                    mybir.AluOpType.add,
                    mybir.AluOpType.max,
                )
                ring.dma_start(out=yb_d.ap()[nt, mb][:, msl], in_=o[:, msl])

            # ---- band 0: k-major waves (8 MMs per arriving chunk) ----
            accs = {nt: ps.tile([P, MB], F32, tag=f"acc{nt}", name=f"acc{nt}") for nt in range(NT)}
            for c in range(KC):
                for nt in GORDER:
                    emit_mm(accs[nt], 0, nt, c, start=(c == 0), stop=(c == KC - 1))
            # evictions in stop order; they overlap band 1's first bursts.
            for i, nt in enumerate(GORDER):
                evict(accs[nt], 0, nt, nc.sync if i % 2 == 0 else nc.scalar)

            # ---- band 1: per-tile k-bursts (stops stagger 1.7us apart) ----
            accs2 = {
                nt: ps.tile([P, MB], F32, tag=f"acc{nt}", name=f"b1acc{nt}")
                for nt in range(NT) if nt != GORDER[-1]
            }
            for i, nt in enumerate(GORDER[:-1]):
                for c in range(KC):
                    emit_mm(accs2[nt], 1, nt, c, start=(c == 0), stop=(c == KC - 1))
                evict(accs2[nt], 1, nt, nc.sync if i % 2 == 0 else nc.scalar)
            # final group: two independent 256-wide half-bursts. The second
            # half borrows the long-free acc7 bank (group 7 ran first), so
            # its matmuls have no dependency on the first half's eviction
            # and the post-last-matmul chain is one short eviction + DMA.
            nt = GORDER[-1]
            for h, (tag, sl) in enumerate(
                (("acc6", slice(0, MB // 2)), ("acc7", slice(MB // 2, MB)))
            ):
                acc_h = ps.tile([P, MB // 2], F32, tag=tag, name=f"b1f{h}")
                for c in range(KC):
                    nc.tensor.matmul(
                        acc_h,
                        w_tiles[c][:, nt * P : (nt + 1) * P],
                        x_tiles[c][:, MB:][:, sl],
                        start=(c == 0),
                        stop=(c == KC - 1),
                    )
                o = outp.tile([P, MB], BF16, tag="otile")
                nc.vector.tensor_scalar(
                    o[:, sl],
                    acc_h,
                    b_sb[:, nt : nt + 1],
                    0.0,
                    mybir.AluOpType.add,
                    mybir.AluOpType.max,
                )
                (nc.scalar if h == 0 else nc.sync).dma_start(
                    out=yb_d.ap()[nt, 1][:, sl], in_=o[:, sl]
                )

    nc.compile()
    return nc


def get_nc():
    if "nc" not in _CACHE:
        _CACHE["nc"] = build_bass()
    return _CACHE["nc"]


def make_in_maps(x, w, b):
    x = np.asarray(x, dtype=np.float32)
    w = np.asarray(w, dtype=np.float32)
    b = np.ascontiguousarray(b, dtype=np.float32)
    w16 = np.ascontiguousarray(w.astype(np.float16))
    b_pc = np.ascontiguousarray(b.reshape(NT, P).T)        # [P, NT]
    xs = x.reshape(N_CORES, M, D_IN)
    maps = []
    for i in range(N_CORES):
        xT = xs[i].T.astype(np.float16)                    # [D_IN, M]
        xblk = np.ascontiguousarray(
            xT.reshape(KC, P, NUM_MB, MB).transpose(0, 2, 1, 3)
        )                                                  # [KC, NUM_MB, P, MB]
        maps.append({"xb": xblk, "w": w16, "b": b_pc})
    return maps


def gather_out(results):
    outs = []
    for i in range(N_CORES):
        yb = results[i]["yb"].astype(np.float32)           # [NT, NUM_MB, P, MB]
        yT = yb.transpose(0, 2, 1, 3).reshape(D_OUT, M)
        outs.append(np.ascontiguousarray(yT.T))
    return np.concatenate(outs, axis=0)


def kernel(x, w, b):
    nc = get_nc()
    res = run_bass_kernel_spmd(nc, make_in_maps(x, w, b), core_ids=list(range(N_CORES)))
    return gather_out(res.results)
